# revision 15
# baseline (speedup 1.0000x reference)
"""ContrastiveLoss Trainium2 kernel (8 NeuronCores, SPMD), v3: symmetric.

Math (reference):
    f = features / ||features||_row            (L2 normalize)
    s_ij = (f_i . f_j) / T,  T = 0.1
    Z_i = sum_{j != i} exp(s_ij)
    per_row_i = (num_pos_i * ln(Z_i) - sum_j mask_ij s_ij) / (num_pos_i + eps)
    loss = mean(per_row)

The similarity matrix is symmetric, so only the "upper triangle" of band
pairs is computed.  Rows are split into 16 bands of 512; core k owns bands
A=k and B=k+8 and computes blocks (A, A+d) for d=0..8 and (B, B+d) for
d=0..7 (mod 16) -- a tournament schedule covering every unordered band pair
exactly once (verified in numpy).  Each core's column buffer is ROTATED by
the host (position t holds global band (k+t) % 16) so the schedule uses
identical compile-time offsets on every core: band A = positions 0..8,
band B = positions 8..15.

Per block: fp8 DoubleRow matmuls -> PSUM, exp on ACT with fused row-sum
accumulation (credits the block's rows).  Off-diagonal blocks additionally
get column sums via ones-matmuls over the bf16 exp tiles (credits the
block's columns, valid because s_ij = s_ji bitwise on the PE).  Host does
layout prep (normalize + fp8 quantize + transpose + rotate) and the O(N)
loss assembly using class sums over the quantized features.
"""

import numpy as np
import ml_dtypes

TEMP_INV = 10.0  # 1/temperature
EPS = 1e-8
N, D, NCORES = 8192, 512, 8
BANDS, BW = 16, 512      # row bands
CG = 2048                # fT column-group tile width
NCG = N // CG            # 4
KC = D // 128            # 4 contraction chunks of 128
FP8_SCALE = 16.0
ACT_SCALE = TEMP_INV / (FP8_SCALE * FP8_SCALE)
CHW = 1536               # main psum/exp chunk width (3 banks)

# (col_start, width) chunks per band: A covers positions 0..8 (4608 cols),
# B covers positions 8..15 (cols 4096..8192).  B's diagonal-only 512-col
# chunk goes LAST: it has no column-sum strips, so the kernel tail is just
# its exp+accum instead of strips waiting on the final exp.
CHUNKS_A = [(0, 1536), (1536, 1536), (3072, 1536)]
CHUNKS_B = [(4608, 1536), (6144, 1536), (7680, 512), (4096, 512)]

_prog_cache = None


def _build_program():
    import concourse.bacc as bacc
    import concourse.tile as tile
    import concourse.hw_specs as hw_specs
    from concourse import mybir

    # Pin the ACT function table set (we only use Exp) so walrus never
    # inserts a mid-kernel ~2.7us table switch.
    tabs = hw_specs.get_activation_tables("gen3")
    keep = "natural_log_exp_and_others"
    if keep in tabs:
        for name in tabs:
            if name != keep:
                tabs[name] = set()

    f32, bf16, fp8 = mybir.dt.float32, mybir.dt.bfloat16, mybir.dt.float8e4
    A = mybir.ActivationFunctionType
    Alu = mybir.AluOpType
    X = mybir.AxisListType.X
    DR = mybir.MatmulPerfMode.DoubleRow

    nc = bacc.Bacc("TRN2", target_bir_lowering=False, debug=False,
                   num_devices=NCORES)

    # featT[cg][p, kc, j] = f8rot[cg*CG + j, kc*128 + p]; rotated columns
    featT = nc.dram_tensor("featT", [NCG, 128, KC, CG], fp8,
                           kind="ExternalInput")
    outz = nc.dram_tensor("outz", [128, 8], f32, kind="ExternalOutput")
    colzd = nc.dram_tensor("colz", [1, 15, 512], f32, kind="ExternalOutput")

    ftv = featT.ap()

    from contextlib import ExitStack

    with tile.TileContext(nc) as tc, ExitStack() as ctx:
        singles = ctx.enter_context(tc.tile_pool(name="singles", bufs=1))
        expp = ctx.enter_context(tc.tile_pool(name="expscr", bufs=8))
        e8p = ctx.enter_context(tc.tile_pool(name="esc8p", bufs=4))

        fT = [singles.tile([128, KC, CG], fp8, tag=f"fT{g}", name=f"fT{g}")
              for g in range(NCG)]
        ones = singles.tile([128, 2, 128], fp8, tag="ones")
        zrows = singles.tile([128, 8, 4], f32, tag="zrows")
        ZE = singles.tile([128, 8], f32, tag="ZE")
        colZ = singles.tile([128, 15, 512], f32, tag="colZ")

        nc.vector.memset(ones, 1.0)
        # serial issue on one queue, in consumption order: each transfer then
        # gets full DMA bandwidth, so early-needed pieces land first (a
        # parallel spread fair-shares bandwidth and stalls the first chunk).
        # Small first piece so the first matmuls start ~1.5us earlier.
        pieces = [(0, 0, 512), (0, 512, 1024), (0, 1536, 512),
                  (1, 0, 1024), (1, 1024, 1024), (2, 0, 1024),
                  (2, 1024, 1024), (3, 0, 1024), (3, 1024, 1024)]
        for g, c0, w in pieces:
            nc.sync.dma_start(out=fT[g][:, :, c0:c0 + w],
                              in_=ftv[g][:, :, c0:c0 + w])

        def rhs_at(col):
            cg, off = col // CG, col % CG
            return fT[cg][:, :, off:off + 512]

        # work list: (band_idx, row_tile_src, chunks, diag_pos)
        work = [(0, fT[0], CHUNKS_A, 0), (1, fT[2], CHUNKS_B, 8)]

        with tc.tile_pool(name="mps", bufs=2, space="PSUM") as mpp, \
                tc.tile_pool(name="sps", bufs=2, space="PSUM") as spp:
            def emit_strips(item):
                # column sums of the exp block: fp8 DoubleRow ones-matmuls
                # over rt pairs (contraction 256 = 2 row tiles per matmul)
                pairs_, c0_, w_, dpos_ = item
                for s in range(w_ // 512):
                    pos = (c0_ + 512 * s) // 512
                    if pos == dpos_:
                        continue
                    st = spp.tile([128, 512], f32, tag="strip")
                    for pr in range(2):
                        nc.tensor.matmul(
                            st, lhsT=ones,
                            rhs=pairs_[pr][:, :, 512 * s:512 * s + 512],
                            start=(pr == 0), stop=(pr == 1), perf_mode=DR)
                    nc.vector.tensor_copy(out=colZ[:, pos - 1], in_=st)

            pending = None  # strips deferred 1 chunk so PE never waits on ACT
            for bi, rowsrc, chunks, dpos in work:
                for ci, (c0, w) in enumerate(chunks):
                    has_strips = any((c0 + 512 * s) // 512 != dpos
                                     for s in range(w // 512))
                    has_diag = any((c0 + 512 * s) // 512 == dpos
                                   for s in range(w // 512))
                    pairs = []
                    for rt in range(4):
                        ps = mpp.tile([128, CHW], f32, tag="ps")
                        for kk in range(2):
                            lhsT = rowsrc[:, 2 * kk:2 * kk + 2,
                                          128 * rt:128 * rt + 128]
                            for ct in range(w // 512):
                                rhs = rhs_at(c0 + 512 * ct)
                                nc.tensor.matmul(
                                    ps[:, 512 * ct:512 * ct + 512],
                                    lhsT=lhsT,
                                    rhs=rhs[:, 2 * kk:2 * kk + 2, :],
                                    start=(kk == 0), stop=(kk == 1),
                                    perf_mode=DR)
                        esc = expp.tile([128, CHW], bf16, tag="esc")
                        # diag chunks keep the f32 ACT accumulator (the e^10
                        # self term needs it); elsewhere row sums come from a
                        # DVE reduce over bf16 esc, saving the ~284ns
                        # READ_ACCUMULATOR on the saturated ACT queue
                        nc.scalar.activation(
                            out=esc[:, :w], in_=ps[:, :w], func=A.Exp,
                            scale=ACT_SCALE,
                            accum_out=(zrows[:, 4 * bi + rt, ci:ci + 1]
                                       if has_diag else None))
                        if not has_diag:
                            nc.vector.tensor_reduce(
                                out=zrows[:, 4 * bi + rt, ci:ci + 1],
                                in_=esc[:, :w], axis=X, op=Alu.add)
                        if has_strips:
                            # cast exp/128 to fp8 on the (idle) vector engine
                            # so strips can use DoubleRow; host scales by 128
                            if rt % 2 == 0:
                                pairs.append(e8p.tile([128, 2, CHW], fp8,
                                                      tag="e8", name="e8"))
                            nc.vector.tensor_scalar(
                                out=pairs[rt // 2][:, rt % 2, :w],
                                in0=esc[:, :w], scalar1=1.0 / 128.0,
                                scalar2=None, op0=Alu.mult)

                    if pending is not None:
                        emit_strips(pending)
                    pending = (pairs, c0, w, dpos) if has_strips else None
            if pending is not None:
                emit_strips(pending)

        for r in range(8):
            nch = len(CHUNKS_A) if r < 4 else len(CHUNKS_B)
            nc.vector.tensor_reduce(out=ZE[:, r:r + 1],
                                    in_=zrows[:, r, 0:nch],
                                    axis=X, op=Alu.add)
        nc.sync.dma_start(out=outz.ap(), in_=ZE)
        nc.gpsimd.dma_start(out=colzd.ap(), in_=colZ[0:1])

    nc.compile()
    return nc


def _get_program():
    global _prog_cache
    if _prog_cache is None:
        _prog_cache = _build_program()
    return _prog_cache


def _prep_inputs(features, labels):
    fp8 = ml_dtypes.float8_e4m3
    f = np.asarray(features, dtype=np.float32)
    lab = np.asarray(labels).astype(np.int64)

    norms = np.sqrt((f.astype(np.float64) ** 2).sum(1))
    norms = np.maximum(norms, 1e-12)
    fhat = f / norms[:, None].astype(np.float32)
    f8 = (fhat * np.float32(FP8_SCALE)).astype(fp8)
    fdq = f8.astype(np.float32)          # what the device actually multiplies
    f8Tb = np.ascontiguousarray(f8.T).reshape(D, BANDS, BW)

    in_maps = []
    for k in range(NCORES):
        order = [(k + t) % BANDS for t in range(BANDS)]
        rot = f8Tb[:, order].reshape(D, N)
        featT_host = np.ascontiguousarray(
            rot.reshape(KC, 128, NCG, CG).transpose(2, 1, 0, 3))
        in_maps.append({"featT": featT_host})

    # host-side loss assembly constants, on the quantized values so they
    # match the device similarity matrix exactly
    fdq64 = fdq.astype(np.float64)
    diag_ss = (fdq64 * fdq64).sum(1)               # f8_i . f8_i
    dexp = np.exp(ACT_SCALE * diag_ss)             # self term inside Z rows
    nclass = int(lab.max()) + 1
    G = np.zeros((nclass, D), np.float64)
    np.add.at(G, lab, fdq64)
    rdq = (fdq64 * G[lab]).sum(1)                  # f8_i . G_{c_i}
    possum = ACT_SCALE * (rdq - diag_ss)           # sum_j mask_ij s_ij
    counts = np.bincount(lab, minlength=nclass)
    npos = (counts[lab] - 1).astype(np.float64)
    aux = dict(dexp=dexp, possum=possum, npos=npos)
    return in_maps, aux


def _run(inputs, trace=False, trace_kwargs=None):
    from concourse.bass_utils import run_bass_kernel_spmd

    nc = _get_program()
    in_maps, aux = _prep_inputs(inputs["features"], inputs["labels"])
    res = run_bass_kernel_spmd(nc, in_maps, core_ids=list(range(NCORES)),
                               trace=trace, **(trace_kwargs or {}))

    Zrow = np.zeros((N,), np.float64)
    Zcol = np.zeros((N,), np.float64)
    for k in range(NCORES):
        out = res.results[k]["outz"].astype(np.float64)   # [128, 8]
        for bi, band in ((0, k), (1, k + 8)):
            for rt in range(4):
                g0 = BW * band + 128 * rt
                Zrow[g0:g0 + 128] = out[:, 4 * bi + rt]
        # colz holds sums of exp/128 (fp8 cast for DoubleRow strips)
        cz = res.results[k]["colz"].astype(np.float64)[0] * 128.0  # [15, 512]
        for t in range(1, BANDS):
            band = (k + t) % BANDS
            Zcol[BW * band:BW * (band + 1)] += cz[t - 1]

    Z = Zrow + Zcol - aux["dexp"]
    per_row = (aux["npos"] * np.log(Z) - aux["possum"]) / (aux["npos"] + EPS)
    loss = np.float32(per_row.mean())
    return loss, res


def kernel(**inputs) -> np.ndarray:
    loss, _ = _run(inputs, trace=False)
    return np.asarray(loss, dtype=np.float32)
